# revision 18
# baseline (speedup 1.0000x reference)
"""AdditiveRelationalGraphConvolution on 8 TRN2 NeuronCores.

out = relu(mean_s(features[neighbors]) @ W.T + mean_s(RWT[relations]))

Data-parallel over batch (4096 rows/core); feature table replicated (bf16).

Neighbor path (per 128-row tile):
  - neighbor rows are fetched with dma_gather (int16 indices). The 100K-row
    table exceeds int16 range, so samples are bucket-sorted (host side) into
    4 static windows; each bucket list is quota-padded (16-granular) with a
    valid dummy index (0). One gather per (tile, bucket) keeps the pipeline
    fine-grained: a tile's matmuls only wait on its own 4 small gathers.
    Gathered slots land at dst[i%128, i//128]; a host-provided per-slot owner
    tag (batch row, or 255 for dead slots) lets the device rebuild one-hot
    selection matrices (DVE is_equal) and aggregate with PE matmuls:
    aggT[i,b] += G[p,i]*sel[p,b]. Slots between quota and the 128-chunk
    boundary are never written by any gather; they are zeroed once at start
    (memset priming of every pool buffer) and masked by dead owners.
  - main transform: psum[b,o] = aggT.T @ (W.T/16).

Relation path: no gather at all. The host histograms relation ids per batch
row (counts <= 16, exact in bf16) and the device computes
  psum[b,o] += sum_r cntT[r,b] * (RWT.T[r,o]/16)
as two K=128 matmuls against the tiny resident relation table.

Relu on ACT, store f32.
"""

import sys

sys.path.insert(0, "/opt/trn_rl_repo")

import numpy as np

N_CORES = 8
B = 32768
S = 16
D = 256
NUM_NODES = 100000
NUM_REL = 238
B_LOC = B // N_CORES  # 4096
P = 128
TILES = B_LOC // P  # 32

# feature-index windows (int16 range, equal sizes so the 4 per-tile gathers
# have matched drain times across the 4 SWDGE queues)
WIN = [(0, 25000), (25000, 50000), (50000, 75000), (75000, 100000)]
DEAD = 255.0
GBUFS = 6  # gather buffers in flight per bucket

_CACHE = {}


def _build(QUOTA):
    import concourse.bass as bass
    import concourse.tile as tile
    from concourse import bacc, mybir

    NCHUNK = [-(-q // P) for q in QUOTA]
    CHUNKS = sum(NCHUNK)
    IDXCOLS = sum(QUOTA) // 16
    f32 = mybir.dt.float32
    bf16 = mybir.dt.bfloat16
    i16 = mybir.dt.int16

    nc = bacc.Bacc(
        "TRN2",
        target_bir_lowering=False,
        debug=False,
        enable_asserts=False,
        num_devices=N_CORES,
        num_swdge_queues=4,
        dynamic_dma_scratch_size=49152,
    )
    feat = nc.dram_tensor("feat", [NUM_NODES, D], bf16, kind="ExternalInput").ap()
    rwts = nc.dram_tensor("rwts", [2 * P, D], bf16, kind="ExternalInput").ap()
    wT = nc.dram_tensor("wT", [D, D], bf16, kind="ExternalInput").ap()
    nidx = nc.dram_tensor("nidx", [P, TILES * IDXCOLS], i16, kind="ExternalInput").ap()
    owner = nc.dram_tensor(
        "owner", [P, TILES * CHUNKS], bf16, kind="ExternalInput"
    ).ap()
    iota = nc.dram_tensor("iota", [P, CHUNKS * P], bf16, kind="ExternalInput").ap()
    cnt = nc.dram_tensor("cnt", [P, TILES * 2 * P], bf16, kind="ExternalInput").ap()
    out = nc.dram_tensor("out", [B_LOC, D], f32, kind="ExternalOutput").ap()

    with tile.TileContext(nc) as tc:
        with (
            tc.tile_pool(name="const", bufs=1) as cp,
            tc.tile_pool(name="gfix", bufs=GBUFS) as gfix,
            tc.tile_pool(name="sel", bufs=4) as selp,
            tc.tile_pool(name="small", bufs=3) as small,
            tc.tile_pool(name="psA", bufs=2, space="PSUM") as psA,
            tc.tile_pool(name="psB", bufs=2, space="PSUM") as psB,
        ):
            # sync (SP) queue: index/owner metadata, first-needed first
            nidx_sb = cp.tile([P, TILES * IDXCOLS], i16)
            NSPLIT = 4
            nw = TILES // NSPLIT * IDXCOLS
            for i in range(NSPLIT):
                nc.sync.dma_start(
                    out=nidx_sb[:, i * nw : (i + 1) * nw],
                    in_=nidx[:, i * nw : (i + 1) * nw],
                )
            owner_sb = cp.tile([P, TILES * CHUNKS], bf16)
            nc.sync.dma_start(out=owner_sb[:], in_=owner[:])
            iota_sb = cp.tile([P, CHUNKS * P], bf16)
            nc.sync.dma_start(out=iota_sb[:], in_=iota[:])
            # ACT queue: matmul constants
            wt_sb = cp.tile([P, 2 * D], bf16)
            nc.scalar.dma_start(out=wt_sb[:, 0:D], in_=wT[0:P, :])
            nc.scalar.dma_start(out=wt_sb[:, D : 2 * D], in_=wT[P : 2 * P, :])
            rwts_sb = cp.tile([P, 2 * D], bf16)
            nc.scalar.dma_start(out=rwts_sb[:, 0:D], in_=rwts[0:P, :])
            nc.scalar.dma_start(out=rwts_sb[:, D : 2 * D], in_=rwts[P : 2 * P, :])
            cnt_sb = cp.tile([P, TILES * 2 * P], bf16)
            nc.scalar.dma_start(out=cnt_sb[:], in_=cnt[:])

            # zero the never-gathered tail slots of every physical buffer once
            for r in range(GBUFS):
                for k in range(4):
                    g = gfix.tile(
                        [P, NCHUNK[k] * D],
                        bf16,
                        name=f"init{r}_{k}",
                        tag=f"gath{k}",
                        bufs=GBUFS,
                    )
                    if QUOTA[k] % P:
                        # whole last chunk: partition sub-ranges hit BIR
                        # partition-offset limits; live slots are rewritten
                        # by every gather anyway
                        nc.vector.memset(
                            g[:, (NCHUNK[k] - 1) * D : NCHUNK[k] * D], 0
                        )

            # sel depends only on owner/iota constants: build it LOOKAHEAD
            # tiles ahead of its consumer so the DVE one-hot build never sits
            # on the per-tile gather->matmul critical chain. LOOKAHEAD must
            # stay below the pool depth (4) so every buffer-reuse WAR edge
            # references an already-emitted reader.
            LOOKAHEAD = 3
            sels = {}

            def emit_sel(t):
                sel = selp.tile([P, CHUNKS * P], bf16, tag="sel", bufs=4)
                ow = owner_sb[:, t * CHUNKS : (t + 1) * CHUNKS]
                nc.vector.tensor_tensor(
                    out=sel[:].rearrange("p (c b) -> p c b", b=P),
                    in0=ow[:, :, None].to_broadcast([P, CHUNKS, P]),
                    in1=iota_sb[:].rearrange("p (c b) -> p c b", b=P),
                    op=mybir.AluOpType.is_equal,
                )
                sels[t] = sel

            for t in range(LOOKAHEAD):
                emit_sel(t)

            aggTs = {}

            def emit_pm(t):
                aggT = aggTs.pop(t)
                pm = psB.tile([P, D], f32, tag="pm", space="PSUM")
                # relation histogram term: pm[b,o] += cntT[r,b]*rwts[r,o]
                nc.tensor.matmul(
                    out=pm[:],
                    lhsT=cnt_sb[:, t * 2 * P : t * 2 * P + P],
                    rhs=rwts_sb[:, 0:D],
                    start=True,
                    stop=False,
                )
                nc.tensor.matmul(
                    out=pm[:],
                    lhsT=cnt_sb[:, t * 2 * P + P : (t + 1) * 2 * P],
                    rhs=rwts_sb[:, D : 2 * D],
                    start=False,
                    stop=False,
                )
                # neighbor term: pm[b,o] += aggT[i,b]*wT[i,o]
                nc.tensor.matmul(
                    out=pm[:],
                    lhsT=aggT[:, 0:P],
                    rhs=wt_sb[:, 0:D],
                    start=False,
                    stop=False,
                )
                nc.tensor.matmul(
                    out=pm[:],
                    lhsT=aggT[:, P : 2 * P],
                    rhs=wt_sb[:, D : 2 * D],
                    start=False,
                    stop=True,
                )
                osb = small.tile([P, D], f32, tag="osb")
                nc.scalar.activation(
                    out=osb[:], in_=pm[:], func=mybir.ActivationFunctionType.Relu
                )
                nc.sync.dma_start(out=out[t * P : (t + 1) * P, :], in_=osb[:])

            for t in range(TILES):
                off = 0
                G = []
                for k in range(4):
                    w = QUOTA[k] // 16
                    g = gfix.tile(
                        [P, NCHUNK[k] * D],
                        bf16,
                        name=f"g{t}_{k}",
                        tag=f"gath{k}",
                        bufs=GBUFS,
                    )
                    nc.gpsimd.dma_gather(
                        out_ap=g[:].rearrange("p (c d) -> p c d", d=D),
                        in_ap=feat[WIN[k][0] : WIN[k][1], :],
                        idxs_ap=nidx_sb[:, t * IDXCOLS + off : t * IDXCOLS + off + w],
                        num_idxs=QUOTA[k],
                        num_idxs_reg=QUOTA[k],
                        elem_size=D,
                        single_packet=False,
                        queue_num=(k + t) % 4,
                    )
                    off += w
                    G.append(g)

                if t + LOOKAHEAD < TILES:
                    emit_sel(t + LOOKAHEAD)
                sel = sels.pop(t)

                # aggT[i, b] = sum_p G[p, i] * sel[p, b] over all nbr chunks
                agT0 = psA.tile([P, P], f32, tag="agT0", space="PSUM")
                agT1 = psA.tile([P, P], f32, tag="agT1", space="PSUM")
                ci = 0
                for k in range(4):
                    for lc in range(NCHUNK[k]):
                        for ic, agT in enumerate((agT0, agT1)):
                            nc.tensor.matmul(
                                out=agT[:],
                                lhsT=G[k][:, lc * D + ic * P : lc * D + (ic + 1) * P],
                                rhs=sel[:, ci * P : (ci + 1) * P],
                                start=(ci == 0),
                                stop=(ci == CHUNKS - 1),
                            )
                        ci += 1
                aggT = small.tile([P, 2 * P], bf16, tag="aggT")
                nc.scalar.activation(
                    out=aggT[:, 0:P],
                    in_=agT0[:],
                    func=mybir.ActivationFunctionType.Copy,
                )
                nc.scalar.activation(
                    out=aggT[:, P : 2 * P],
                    in_=agT1[:],
                    func=mybir.ActivationFunctionType.Copy,
                )
                aggTs[t] = aggT

                # software-pipelined: tile t-1's output block sits behind
                # tile t's agg matmuls so the PE never idles waiting for the
                # ACT psum->sbuf copies of the current tile
                if t >= 1:
                    emit_pm(t - 1)
            emit_pm(TILES - 1)
    nc.compile()
    return nc


def _get_nc(QUOTA):
    key = ("nc", tuple(QUOTA))
    if key not in _CACHE:
        _CACHE[key] = _build(tuple(QUOTA))
    return _CACHE[key]


def _wrap16(lst, width):
    """Wrap a flat ALL-VALID index list of length width*16 into [128, width]
    int16 (16-partition wrap, replicated to all 8 gpsimd core groups).
    Negative indices + multiple in-flight gathers crash the Q7 ucode, so
    callers must pad with a valid dummy index instead."""
    n = len(lst)
    assert n == width * 16
    outw = np.asarray(lst, dtype=np.int16).reshape(width, 16).T
    return np.tile(outw, (8, 1))


def _quotas_for(neighbors):
    """Smallest 16-multiple quota per bucket covering the actual input."""
    nb = np.ascontiguousarray(neighbors, dtype=np.int64).reshape(
        N_CORES * TILES, P * S
    )
    quotas = []
    for k in range(4):
        cnts = ((nb >= WIN[k][0]) & (nb < WIN[k][1])).sum(axis=1)
        q = int(-(-max(1, cnts.max()) // 16) * 16)
        quotas.append(q)
    return tuple(quotas)


def _prep_inputs(neighbors, relations, features, weight, relation_weight, QUOTA):
    import ml_dtypes

    NCHUNK = [-(-q // P) for q in QUOTA]
    CHUNKS = sum(NCHUNK)
    IDXCOLS = sum(QUOTA) // 16

    bf16 = ml_dtypes.bfloat16
    inv_s = np.float32(1.0 / S)

    nb = np.ascontiguousarray(neighbors, dtype=np.int64).reshape(N_CORES, TILES, P, S)
    rl = np.ascontiguousarray(relations, dtype=np.int64).reshape(
        N_CORES, TILES * P, S
    )
    feat = np.ascontiguousarray(features.astype(bf16))
    rwts_f = np.zeros((2 * P, D), dtype=np.float32)
    rwts_f[:NUM_REL] = relation_weight.T.astype(np.float32) * inv_s
    rwts = np.ascontiguousarray(rwts_f.astype(bf16))
    wT = np.ascontiguousarray((weight.T.astype(np.float32) * inv_s).astype(bf16))
    iota = np.ascontiguousarray(
        np.tile(np.arange(P, dtype=np.float32), (P, CHUNKS)).astype(bf16)
    )

    in_maps = []
    for core in range(N_CORES):
        nidx = np.zeros((P, TILES * IDXCOLS), dtype=np.int16)
        owner = np.full((P, TILES * CHUNKS), DEAD, dtype=np.float32)
        for t in range(TILES):
            idxs = nb[core, t].ravel()  # j = b*16+s
            owners_flat = np.repeat(np.arange(P), S)
            goff = 0
            cbase = 0
            for k in range(4):
                m = (idxs >= WIN[k][0]) & (idxs < WIN[k][1])
                li = idxs[m] - WIN[k][0]
                lo = owners_flat[m]
                order = np.argsort(li, kind="stable")  # ascending HBM addresses
                li = li[order]
                lo = lo[order]
                cnt_k = len(li)
                assert cnt_k <= QUOTA[k], f"bucket {k} overflow: {cnt_k} > {QUOTA[k]}"
                w = QUOTA[k] // 16
                lpad = np.zeros(QUOTA[k], dtype=np.int16)
                lpad[:cnt_k] = li
                c0 = t * IDXCOLS + goff
                nidx[:, c0 : c0 + w] = _wrap16(lpad, w)
                goff += w
                # owner per slot: slot i -> (p=i%128, chunk=i//128)
                ow = np.full(NCHUNK[k] * P, DEAD, dtype=np.float32)
                ow[:cnt_k] = lo
                owner[
                    :, t * CHUNKS + cbase : t * CHUNKS + cbase + NCHUNK[k]
                ] = ow.reshape(NCHUNK[k], P).T
                cbase += NCHUNK[k]
        # relation histogram: cnt[p, t*256 + k*128 + b] = #{s: rl[t*128+b, s] == k*128+p}
        flat = (
            np.arange(TILES * P, dtype=np.int64)[:, None] * (NUM_REL + 1)
            + rl[core]
        ).ravel()
        counts = np.bincount(flat, minlength=TILES * P * (NUM_REL + 1)).reshape(
            TILES * P, NUM_REL + 1
        )
        arrp = np.zeros((TILES, P, 2 * P), dtype=np.float32)
        arrp[:, :, : NUM_REL + 1] = counts.reshape(TILES, P, NUM_REL + 1)
        cnt_host = (
            arrp.reshape(TILES, P, 2, P)
            .transpose(3, 0, 2, 1)
            .reshape(P, TILES * 2 * P)
        )
        in_maps.append(
            {
                "feat": feat,
                "rwts": rwts,
                "wT": wT,
                "nidx": nidx,
                "owner": np.ascontiguousarray(owner.astype(bf16)),
                "iota": iota,
                "cnt": np.ascontiguousarray(cnt_host.astype(bf16)),
            }
        )
    return in_maps


def run(in_maps, QUOTA, trace=False, tmpdir=None):
    from concourse.bass_utils import run_bass_kernel_spmd

    nc = _get_nc(QUOTA)
    res = run_bass_kernel_spmd(
        nc, in_maps, core_ids=list(range(N_CORES)), trace=trace, tmpdir=tmpdir
    )
    out = np.concatenate([res.results[i]["out"] for i in range(N_CORES)], axis=0)
    return out, res


def kernel(neighbors, relations, features, weight, relation_weight):
    QUOTA = _quotas_for(neighbors)
    in_maps = _prep_inputs(
        neighbors, relations, features, weight, relation_weight, QUOTA
    )
    out, _ = run(in_maps, QUOTA, trace=False)
    return out


# revision 19
# speedup vs baseline: 1.1315x; 1.1315x over previous
"""AdditiveRelationalGraphConvolution on 8 TRN2 NeuronCores.

out = relu(mean_s(features[neighbors]) @ W.T + mean_s(RWT[relations]))

Data-parallel over batch (4096 rows/core); feature table replicated (bf16).

Neighbor path (per 128-row tile):
  - neighbor rows are fetched with dma_gather (int16 indices). The 100K-row
    table exceeds int16 range, so samples are bucket-sorted (host side) into
    4 static windows; each bucket list is quota-padded (16-granular) with a
    valid dummy index (0). One gather per (tile, bucket) keeps the pipeline
    fine-grained: a tile's matmuls only wait on its own 4 small gathers.
    Gathered slots land at dst[i%128, i//128]; a host-provided per-slot owner
    tag (batch row, or 255 for dead slots) lets the device rebuild one-hot
    selection matrices (DVE is_equal) and aggregate with PE matmuls:
    aggT[i,b] += G[p,i]*sel[p,b]. Slots between quota and the 128-chunk
    boundary are never written by any gather; they are zeroed once at start
    (memset priming of every pool buffer) and masked by dead owners.
  - main transform: psum[b,o] = aggT.T @ (W.T/16).

Relation path: no gather at all. The host histograms relation ids per batch
row (counts <= 16, exact in bf16) and the device computes
  psum[b,o] += sum_r cntT[r,b] * (RWT.T[r,o]/16)
as two K=128 matmuls against the tiny resident relation table.

Relu on ACT, store f32.
"""

import sys

sys.path.insert(0, "/opt/trn_rl_repo")

import numpy as np

N_CORES = 8
B = 32768
S = 16
D = 256
NUM_NODES = 100000
NUM_REL = 238
B_LOC = B // N_CORES  # 4096
P = 128
TILES = B_LOC // P  # 32

# feature-index windows (int16 range, equal sizes so the 4 per-tile gathers
# have matched drain times across the 4 SWDGE queues)
WIN = [(0, 25000), (25000, 50000), (50000, 75000), (75000, 100000)]
DEAD = 255.0
GBUFS = 6  # gather buffers in flight per bucket

_CACHE = {}


def _build(QUOTA):
    import concourse.bass as bass
    import concourse.tile as tile
    from concourse import bacc, mybir

    NCHUNK = [-(-q // P) for q in QUOTA]
    CHUNKS = sum(NCHUNK)
    IDXCOLS = sum(QUOTA) // 16
    f32 = mybir.dt.float32
    bf16 = mybir.dt.bfloat16
    i16 = mybir.dt.int16

    nc = bacc.Bacc(
        "TRN2",
        target_bir_lowering=False,
        debug=False,
        enable_asserts=False,
        num_devices=N_CORES,
        num_swdge_queues=4,
        dynamic_dma_scratch_size=49152,
    )
    feat = nc.dram_tensor("feat", [NUM_NODES, D], bf16, kind="ExternalInput").ap()
    rwts = nc.dram_tensor("rwts", [2 * P, D], bf16, kind="ExternalInput").ap()
    wT = nc.dram_tensor("wT", [D, D], bf16, kind="ExternalInput").ap()
    nidx = nc.dram_tensor("nidx", [P, TILES * IDXCOLS], i16, kind="ExternalInput").ap()
    owner = nc.dram_tensor(
        "owner", [P, TILES * CHUNKS], bf16, kind="ExternalInput"
    ).ap()
    iota = nc.dram_tensor("iota", [P, CHUNKS * P], bf16, kind="ExternalInput").ap()
    cnt = nc.dram_tensor("cnt", [P, TILES * 2 * P], bf16, kind="ExternalInput").ap()
    out = nc.dram_tensor("out", [B_LOC, D], f32, kind="ExternalOutput").ap()

    with tile.TileContext(nc) as tc:
        with (
            tc.tile_pool(name="const", bufs=1) as cp,
            tc.tile_pool(name="gfix", bufs=GBUFS) as gfix,
            tc.tile_pool(name="sel", bufs=4) as selp,
            tc.tile_pool(name="small", bufs=3) as small,
            tc.tile_pool(name="psA", bufs=2, space="PSUM") as psA,
            tc.tile_pool(name="psB", bufs=2, space="PSUM") as psB,
        ):
            # sync (SP) queue: index/owner metadata, first-needed first
            nidx_sb = cp.tile([P, TILES * IDXCOLS], i16)
            NSPLIT = 4
            nw = TILES // NSPLIT * IDXCOLS
            for i in range(NSPLIT):
                nc.sync.dma_start(
                    out=nidx_sb[:, i * nw : (i + 1) * nw],
                    in_=nidx[:, i * nw : (i + 1) * nw],
                )
            owner_sb = cp.tile([P, TILES * CHUNKS], bf16)
            nc.sync.dma_start(out=owner_sb[:], in_=owner[:])
            iota_sb = cp.tile([P, CHUNKS * P], bf16)
            nc.sync.dma_start(out=iota_sb[:], in_=iota[:])
            # ACT queue: matmul constants
            wt_sb = cp.tile([P, 2 * D], bf16)
            nc.scalar.dma_start(out=wt_sb[:, 0:D], in_=wT[0:P, :])
            nc.scalar.dma_start(out=wt_sb[:, D : 2 * D], in_=wT[P : 2 * P, :])
            rwts_sb = cp.tile([P, 2 * D], bf16)
            nc.scalar.dma_start(out=rwts_sb[:, 0:D], in_=rwts[0:P, :])
            nc.scalar.dma_start(out=rwts_sb[:, D : 2 * D], in_=rwts[P : 2 * P, :])
            cnt_sb = cp.tile([P, TILES * 2 * P], bf16)
            nc.scalar.dma_start(out=cnt_sb[:], in_=cnt[:])

            # zero the never-gathered tail slots of every physical buffer once
            for r in range(GBUFS):
                for k in range(4):
                    g = gfix.tile(
                        [P, NCHUNK[k] * D],
                        bf16,
                        name=f"init{r}_{k}",
                        tag=f"gath{k}",
                        bufs=GBUFS,
                    )
                    if QUOTA[k] % P:
                        # whole last chunk: partition sub-ranges hit BIR
                        # partition-offset limits; live slots are rewritten
                        # by every gather anyway
                        nc.vector.memset(
                            g[:, (NCHUNK[k] - 1) * D : NCHUNK[k] * D], 0
                        )

            # sel depends only on owner/iota constants: build it LOOKAHEAD
            # tiles ahead of its consumer so the DVE one-hot build never sits
            # on the per-tile gather->matmul critical chain. LOOKAHEAD must
            # stay below the pool depth (4) so every buffer-reuse WAR edge
            # references an already-emitted reader.
            LOOKAHEAD = 3
            sels = {}

            def emit_sel(t):
                sel = selp.tile([P, CHUNKS * P], bf16, tag="sel", bufs=4)
                ow = owner_sb[:, t * CHUNKS : (t + 1) * CHUNKS]
                nc.vector.tensor_tensor(
                    out=sel[:].rearrange("p (c b) -> p c b", b=P),
                    in0=ow[:, :, None].to_broadcast([P, CHUNKS, P]),
                    in1=iota_sb[:].rearrange("p (c b) -> p c b", b=P),
                    op=mybir.AluOpType.is_equal,
                )
                sels[t] = sel

            for t in range(LOOKAHEAD):
                emit_sel(t)

            for t in range(TILES):
                off = 0
                G = []
                for k in range(4):
                    w = QUOTA[k] // 16
                    g = gfix.tile(
                        [P, NCHUNK[k] * D],
                        bf16,
                        name=f"g{t}_{k}",
                        tag=f"gath{k}",
                        bufs=GBUFS,
                    )
                    nc.gpsimd.dma_gather(
                        out_ap=g[:].rearrange("p (c d) -> p c d", d=D),
                        in_ap=feat[WIN[k][0] : WIN[k][1], :],
                        idxs_ap=nidx_sb[:, t * IDXCOLS + off : t * IDXCOLS + off + w],
                        num_idxs=QUOTA[k],
                        num_idxs_reg=QUOTA[k],
                        elem_size=D,
                        single_packet=False,
                        queue_num=(k + t) % 4,
                    )
                    off += w
                    G.append(g)

                if t + LOOKAHEAD < TILES:
                    emit_sel(t + LOOKAHEAD)
                sel = sels.pop(t)

                # aggT[i, b] = sum_p G[p, i] * sel[p, b] over all nbr chunks
                agT0 = psA.tile([P, P], f32, tag="agT0", space="PSUM")
                agT1 = psA.tile([P, P], f32, tag="agT1", space="PSUM")
                ci = 0
                for k in range(4):
                    for lc in range(NCHUNK[k]):
                        for ic, agT in enumerate((agT0, agT1)):
                            nc.tensor.matmul(
                                out=agT[:],
                                lhsT=G[k][:, lc * D + ic * P : lc * D + (ic + 1) * P],
                                rhs=sel[:, ci * P : (ci + 1) * P],
                                start=(ci == 0),
                                stop=(ci == CHUNKS - 1),
                            )
                        ci += 1
                aggT = small.tile([P, 2 * P], bf16, tag="aggT")
                nc.scalar.activation(
                    out=aggT[:, 0:P],
                    in_=agT0[:],
                    func=mybir.ActivationFunctionType.Copy,
                )
                nc.scalar.activation(
                    out=aggT[:, P : 2 * P],
                    in_=agT1[:],
                    func=mybir.ActivationFunctionType.Copy,
                )

                pm = psB.tile([P, D], f32, tag="pm", space="PSUM")
                # relation histogram term: pm[b,o] += cntT[r,b]*rwts[r,o]
                nc.tensor.matmul(
                    out=pm[:],
                    lhsT=cnt_sb[:, t * 2 * P : t * 2 * P + P],
                    rhs=rwts_sb[:, 0:D],
                    start=True,
                    stop=False,
                )
                nc.tensor.matmul(
                    out=pm[:],
                    lhsT=cnt_sb[:, t * 2 * P + P : (t + 1) * 2 * P],
                    rhs=rwts_sb[:, D : 2 * D],
                    start=False,
                    stop=False,
                )
                # neighbor term: pm[b,o] += aggT[i,b]*wT[i,o]
                nc.tensor.matmul(
                    out=pm[:],
                    lhsT=aggT[:, 0:P],
                    rhs=wt_sb[:, 0:D],
                    start=False,
                    stop=False,
                )
                nc.tensor.matmul(
                    out=pm[:],
                    lhsT=aggT[:, P : 2 * P],
                    rhs=wt_sb[:, D : 2 * D],
                    start=False,
                    stop=True,
                )
                osb = small.tile([P, D], f32, tag="osb")
                nc.scalar.activation(
                    out=osb[:], in_=pm[:], func=mybir.ActivationFunctionType.Relu
                )
                nc.sync.dma_start(out=out[t * P : (t + 1) * P, :], in_=osb[:])
    nc.compile()
    return nc


def _get_nc(QUOTA):
    key = ("nc", tuple(QUOTA))
    if key not in _CACHE:
        _CACHE[key] = _build(tuple(QUOTA))
    return _CACHE[key]


def _wrap16(lst, width):
    """Wrap a flat ALL-VALID index list of length width*16 into [128, width]
    int16 (16-partition wrap, replicated to all 8 gpsimd core groups).
    Negative indices + multiple in-flight gathers crash the Q7 ucode, so
    callers must pad with a valid dummy index instead."""
    n = len(lst)
    assert n == width * 16
    outw = np.asarray(lst, dtype=np.int16).reshape(width, 16).T
    return np.tile(outw, (8, 1))


def _quotas_for(neighbors):
    """Smallest 16-multiple quota per bucket covering the actual input."""
    nb = np.ascontiguousarray(neighbors, dtype=np.int64).reshape(
        N_CORES * TILES, P * S
    )
    quotas = []
    for k in range(4):
        cnts = ((nb >= WIN[k][0]) & (nb < WIN[k][1])).sum(axis=1)
        q = int(-(-max(1, cnts.max()) // 16) * 16)
        quotas.append(q)
    return tuple(quotas)


def _prep_inputs(neighbors, relations, features, weight, relation_weight, QUOTA):
    import ml_dtypes

    NCHUNK = [-(-q // P) for q in QUOTA]
    CHUNKS = sum(NCHUNK)
    IDXCOLS = sum(QUOTA) // 16

    bf16 = ml_dtypes.bfloat16
    inv_s = np.float32(1.0 / S)

    nb = np.ascontiguousarray(neighbors, dtype=np.int64).reshape(N_CORES, TILES, P, S)
    rl = np.ascontiguousarray(relations, dtype=np.int64).reshape(
        N_CORES, TILES * P, S
    )
    feat = np.ascontiguousarray(features.astype(bf16))
    rwts_f = np.zeros((2 * P, D), dtype=np.float32)
    rwts_f[:NUM_REL] = relation_weight.T.astype(np.float32) * inv_s
    rwts = np.ascontiguousarray(rwts_f.astype(bf16))
    wT = np.ascontiguousarray((weight.T.astype(np.float32) * inv_s).astype(bf16))
    iota = np.ascontiguousarray(
        np.tile(np.arange(P, dtype=np.float32), (P, CHUNKS)).astype(bf16)
    )

    in_maps = []
    for core in range(N_CORES):
        nidx = np.zeros((P, TILES * IDXCOLS), dtype=np.int16)
        owner = np.full((P, TILES * CHUNKS), DEAD, dtype=np.float32)
        for t in range(TILES):
            idxs = nb[core, t].ravel()  # j = b*16+s
            owners_flat = np.repeat(np.arange(P), S)
            goff = 0
            cbase = 0
            for k in range(4):
                m = (idxs >= WIN[k][0]) & (idxs < WIN[k][1])
                li = idxs[m] - WIN[k][0]
                lo = owners_flat[m]
                order = np.argsort(li, kind="stable")  # ascending HBM addresses
                li = li[order]
                lo = lo[order]
                cnt_k = len(li)
                assert cnt_k <= QUOTA[k], f"bucket {k} overflow: {cnt_k} > {QUOTA[k]}"
                w = QUOTA[k] // 16
                lpad = np.zeros(QUOTA[k], dtype=np.int16)
                lpad[:cnt_k] = li
                c0 = t * IDXCOLS + goff
                nidx[:, c0 : c0 + w] = _wrap16(lpad, w)
                goff += w
                # owner per slot: slot i -> (p=i%128, chunk=i//128)
                ow = np.full(NCHUNK[k] * P, DEAD, dtype=np.float32)
                ow[:cnt_k] = lo
                owner[
                    :, t * CHUNKS + cbase : t * CHUNKS + cbase + NCHUNK[k]
                ] = ow.reshape(NCHUNK[k], P).T
                cbase += NCHUNK[k]
        # relation histogram: cnt[p, t*256 + k*128 + b] = #{s: rl[t*128+b, s] == k*128+p}
        flat = (
            np.arange(TILES * P, dtype=np.int64)[:, None] * (NUM_REL + 1)
            + rl[core]
        ).ravel()
        counts = np.bincount(flat, minlength=TILES * P * (NUM_REL + 1)).reshape(
            TILES * P, NUM_REL + 1
        )
        arrp = np.zeros((TILES, P, 2 * P), dtype=np.float32)
        arrp[:, :, : NUM_REL + 1] = counts.reshape(TILES, P, NUM_REL + 1)
        cnt_host = (
            arrp.reshape(TILES, P, 2, P)
            .transpose(3, 0, 2, 1)
            .reshape(P, TILES * 2 * P)
        )
        in_maps.append(
            {
                "feat": feat,
                "rwts": rwts,
                "wT": wT,
                "nidx": nidx,
                "owner": np.ascontiguousarray(owner.astype(bf16)),
                "iota": iota,
                "cnt": np.ascontiguousarray(cnt_host.astype(bf16)),
            }
        )
    return in_maps


def run(in_maps, QUOTA, trace=False, tmpdir=None):
    from concourse.bass_utils import run_bass_kernel_spmd

    nc = _get_nc(QUOTA)
    res = run_bass_kernel_spmd(
        nc, in_maps, core_ids=list(range(N_CORES)), trace=trace, tmpdir=tmpdir
    )
    out = np.concatenate([res.results[i]["out"] for i in range(N_CORES)], axis=0)
    return out, res


def kernel(neighbors, relations, features, weight, relation_weight):
    QUOTA = _quotas_for(neighbors)
    in_maps = _prep_inputs(
        neighbors, relations, features, weight, relation_weight, QUOTA
    )
    out, _ = run(in_maps, QUOTA, trace=False)
    return out


# revision 27
# speedup vs baseline: 1.1898x; 1.0515x over previous
"""AdditiveRelationalGraphConvolution on 8 TRN2 NeuronCores.

out = relu(mean_s(features[neighbors]) @ W.T + mean_s(RWT[relations]))

Data-parallel over batch (4096 rows/core); feature table replicated (bf16).

Neighbor path (per 128-row tile):
  - neighbor rows are fetched with dma_gather (int16 indices). The 100K-row
    table exceeds int16 range, so samples are bucket-sorted (host side) into
    4 static windows; each bucket list is quota-padded (16-granular) with a
    valid dummy index (0). One gather per (tile, bucket) keeps the pipeline
    fine-grained: a tile's matmuls only wait on its own 4 small gathers.
    Gathered slots land at dst[i%128, i//128]; a host-provided per-slot owner
    tag (batch row, or 255 for dead slots) lets the device rebuild one-hot
    selection matrices (DVE is_equal) and aggregate with PE matmuls:
    aggT[i,b] += G[p,i]*sel[p,b]. Slots between quota and the 128-chunk
    boundary are never written by any gather; they are zeroed once at start
    (memset priming of every pool buffer) and masked by dead owners.
  - main transform: psum[b,o] = aggT.T @ (W.T/16).

Relation path: no gather at all. The host histograms relation ids per batch
row (counts <= 16, exact in bf16) and the device computes
  psum[b,o] += sum_r cntT[r,b] * (RWT.T[r,o]/16)
as two K=128 matmuls against the tiny resident relation table.

Relu on ACT, store f32.
"""

import sys

sys.path.insert(0, "/opt/trn_rl_repo")

import numpy as np

N_CORES = 8
B = 32768
S = 16
D = 256
NUM_NODES = 100000
NUM_REL = 238
B_LOC = B // N_CORES  # 4096
P = 128
TILES = B_LOC // P  # 32

# feature-index windows (int16 range, equal sizes so the 4 per-tile gathers
# have matched drain times across the 4 SWDGE queues)
WIN = [(0, 25000), (25000, 50000), (50000, 75000), (75000, 100000)]
DEAD = 255.0
GBUFS = 6  # gather buffers in flight per bucket

_CACHE = {}


def _build(QUOTA):
    import concourse.bass as bass
    import concourse.tile as tile
    from concourse import bacc, mybir

    NCHUNK = [-(-q // P) for q in QUOTA]
    CHUNKS = sum(NCHUNK)
    IDXCOLS = sum(QUOTA) // 16
    f32 = mybir.dt.float32
    bf16 = mybir.dt.bfloat16
    i16 = mybir.dt.int16

    nc = bacc.Bacc(
        "TRN2",
        target_bir_lowering=False,
        debug=False,
        enable_asserts=False,
        num_devices=N_CORES,
        num_swdge_queues=4,
        dynamic_dma_scratch_size=49152,
    )
    feat = nc.dram_tensor("feat", [NUM_NODES, D], bf16, kind="ExternalInput").ap()
    rwts = nc.dram_tensor("rwts", [2 * P, D], bf16, kind="ExternalInput").ap()
    wT = nc.dram_tensor("wT", [D, D], bf16, kind="ExternalInput").ap()
    nidx = nc.dram_tensor("nidx", [P, TILES * IDXCOLS], i16, kind="ExternalInput").ap()
    owner = nc.dram_tensor(
        "owner", [P, TILES * CHUNKS], bf16, kind="ExternalInput"
    ).ap()
    iota = nc.dram_tensor("iota", [P, CHUNKS * P], bf16, kind="ExternalInput").ap()
    cnt = nc.dram_tensor("cnt", [P, TILES * 2 * P], bf16, kind="ExternalInput").ap()
    out = nc.dram_tensor("out", [B_LOC, D], f32, kind="ExternalOutput").ap()

    with tile.TileContext(nc) as tc:
        with (
            tc.tile_pool(name="const", bufs=1) as cp,
            tc.tile_pool(name="gfix", bufs=GBUFS) as gfix,
            tc.tile_pool(name="sel", bufs=4) as selp,
            tc.tile_pool(name="small", bufs=3) as small,
            tc.tile_pool(name="psA", bufs=2, space="PSUM") as psA,
            tc.tile_pool(name="psB", bufs=2, space="PSUM") as psB,
        ):
            # sync (SP) queue: index/owner metadata, first-needed first
            nidx_sb = cp.tile([P, TILES * IDXCOLS], i16)
            NSPLIT = 4
            nw = TILES // NSPLIT * IDXCOLS
            for i in range(NSPLIT):
                nc.sync.dma_start(
                    out=nidx_sb[:, i * nw : (i + 1) * nw],
                    in_=nidx[:, i * nw : (i + 1) * nw],
                )
            owner_sb = cp.tile([P, TILES * CHUNKS], bf16)
            nc.sync.dma_start(out=owner_sb[:], in_=owner[:])
            iota_sb = cp.tile([P, CHUNKS * P], bf16)
            nc.sync.dma_start(out=iota_sb[:], in_=iota[:])
            # ACT queue: matmul constants
            wt_sb = cp.tile([P, 2 * D], bf16)
            nc.scalar.dma_start(out=wt_sb[:, 0:D], in_=wT[0:P, :])
            nc.scalar.dma_start(out=wt_sb[:, D : 2 * D], in_=wT[P : 2 * P, :])
            rwts_sb = cp.tile([P, 2 * D], bf16)
            nc.scalar.dma_start(out=rwts_sb[:, 0:D], in_=rwts[0:P, :])
            nc.scalar.dma_start(out=rwts_sb[:, D : 2 * D], in_=rwts[P : 2 * P, :])
            cnt_sb = cp.tile([P, TILES * 2 * P], bf16)
            nc.scalar.dma_start(out=cnt_sb[:], in_=cnt[:])

            # zero the never-gathered tail slots of every physical buffer once
            for r in range(GBUFS):
                for k in range(4):
                    g = gfix.tile(
                        [P, NCHUNK[k] * D],
                        bf16,
                        name=f"init{r}_{k}",
                        tag=f"gath{k}",
                        bufs=GBUFS,
                    )
                    if QUOTA[k] % P:
                        # whole last chunk: partition sub-ranges hit BIR
                        # partition-offset limits; live slots are rewritten
                        # by every gather anyway
                        nc.vector.memset(
                            g[:, (NCHUNK[k] - 1) * D : NCHUNK[k] * D], 0
                        )

            # sel depends only on owner/iota constants: build it LOOKAHEAD
            # tiles ahead of its consumer so the DVE one-hot build never sits
            # on the per-tile gather->matmul critical chain. LOOKAHEAD must
            # stay below the pool depth (4) so every buffer-reuse WAR edge
            # references an already-emitted reader.
            LOOKAHEAD = 3
            sels = {}

            def emit_sel(t):
                sel = selp.tile([P, CHUNKS * P], bf16, tag="sel", bufs=4)
                ow = owner_sb[:, t * CHUNKS : (t + 1) * CHUNKS]
                nc.vector.tensor_tensor(
                    out=sel[:].rearrange("p (c b) -> p c b", b=P),
                    in0=ow[:, :, None].to_broadcast([P, CHUNKS, P]),
                    in1=iota_sb[:].rearrange("p (c b) -> p c b", b=P),
                    op=mybir.AluOpType.is_equal,
                )
                sels[t] = sel

            for t in range(LOOKAHEAD):
                emit_sel(t)

            for t in range(TILES):
                off = 0
                G = []
                for k in range(4):
                    w = QUOTA[k] // 16
                    g = gfix.tile(
                        [P, NCHUNK[k] * D],
                        bf16,
                        name=f"g{t}_{k}",
                        tag=f"gath{k}",
                        bufs=GBUFS,
                    )
                    nc.gpsimd.dma_gather(
                        out_ap=g[:].rearrange("p (c d) -> p c d", d=D),
                        in_ap=feat[WIN[k][0] : WIN[k][1], :],
                        idxs_ap=nidx_sb[:, t * IDXCOLS + off : t * IDXCOLS + off + w],
                        num_idxs=QUOTA[k],
                        num_idxs_reg=QUOTA[k],
                        elem_size=D,
                        single_packet=False,
                        queue_num=(k + t) % 4,
                    )
                    off += w
                    G.append(g)

                if t + LOOKAHEAD < TILES:
                    emit_sel(t + LOOKAHEAD)
                sel = sels.pop(t)

                # aggT[i, b] = sum_p G[p, i] * sel[p, b] over all nbr chunks
                agT0 = psA.tile([P, P], f32, tag="agT0", space="PSUM")
                agT1 = psA.tile([P, P], f32, tag="agT1", space="PSUM")
                ci = 0
                for k in range(4):
                    for lc in range(NCHUNK[k]):
                        for ic, agT in enumerate((agT0, agT1)):
                            nc.tensor.matmul(
                                out=agT[:],
                                lhsT=G[k][:, lc * D + ic * P : lc * D + (ic + 1) * P],
                                rhs=sel[:, ci * P : (ci + 1) * P],
                                start=(ci == 0),
                                stop=(ci == CHUNKS - 1),
                            )
                        ci += 1
                aggT = small.tile([P, 2 * P], bf16, tag="aggT")
                nc.scalar.activation(
                    out=aggT[:, 0:P],
                    in_=agT0[:],
                    func=mybir.ActivationFunctionType.Copy,
                )
                nc.scalar.activation(
                    out=aggT[:, P : 2 * P],
                    in_=agT1[:],
                    func=mybir.ActivationFunctionType.Copy,
                )

                pm = psB.tile([P, D], f32, tag="pm", space="PSUM")
                # relation histogram term: pm[b,o] += cntT[r,b]*rwts[r,o]
                nc.tensor.matmul(
                    out=pm[:],
                    lhsT=cnt_sb[:, t * 2 * P : t * 2 * P + P],
                    rhs=rwts_sb[:, 0:D],
                    start=True,
                    stop=False,
                )
                nc.tensor.matmul(
                    out=pm[:],
                    lhsT=cnt_sb[:, t * 2 * P + P : (t + 1) * 2 * P],
                    rhs=rwts_sb[:, D : 2 * D],
                    start=False,
                    stop=False,
                )
                # neighbor term: pm[b,o] += aggT[i,b]*wT[i,o]
                nc.tensor.matmul(
                    out=pm[:],
                    lhsT=aggT[:, 0:P],
                    rhs=wt_sb[:, 0:D],
                    start=False,
                    stop=False,
                )
                nc.tensor.matmul(
                    out=pm[:],
                    lhsT=aggT[:, P : 2 * P],
                    rhs=wt_sb[:, D : 2 * D],
                    start=False,
                    stop=True,
                )
                osb = small.tile([P, D], f32, tag="osb")
                nc.scalar.activation(
                    out=osb[:], in_=pm[:], func=mybir.ActivationFunctionType.Relu
                )
                nc.sync.dma_start(out=out[t * P : (t + 1) * P, :], in_=osb[:])
    nc.compile()
    return nc


def _get_nc(QUOTA):
    key = ("nc", tuple(QUOTA))
    if key not in _CACHE:
        _CACHE[key] = _build(tuple(QUOTA))
    return _CACHE[key]


def _wrap16(lst, width):
    """Wrap a flat ALL-VALID index list of length width*16 into [128, width]
    int16 (16-partition wrap, replicated to all 8 gpsimd core groups).
    Negative indices + multiple in-flight gathers crash the Q7 ucode, so
    callers must pad with a valid dummy index instead."""
    n = len(lst)
    assert n == width * 16
    outw = np.asarray(lst, dtype=np.int16).reshape(width, 16).T
    return np.tile(outw, (8, 1))


def _balance_perm(neighbors):
    """Permute batch rows across the 256 (core, tile) bins so per-bin bucket
    counts flatten toward the mean: gather time scales with the max bucket
    count over bins (the static quota), so balancing shrinks every quota.
    Greedy: place the most skewed rows first, each into the bin whose
    resulting max bucket load is smallest."""
    nb = np.ascontiguousarray(neighbors, dtype=np.int64)
    cnts = np.stack(
        [((nb >= WIN[k][0]) & (nb < WIN[k][1])).sum(1) for k in range(4)], 1
    ).astype(np.int32)
    NB = N_CORES * TILES
    order = np.argsort(-cnts.max(1), kind="stable")
    loads = np.zeros((NB, 4), dtype=np.int32)
    fill = np.zeros(NB, dtype=np.int32)
    bins = [[] for _ in range(NB)]
    for r in order:
        cand = (loads + cnts[r]).max(1).astype(np.float64)
        cand[fill >= P] = np.inf
        j = int(cand.argmin())
        bins[j].append(r)
        loads[j] += cnts[r]
        fill[j] += 1
    return np.concatenate([np.asarray(b, dtype=np.int64) for b in bins])


def _quotas_for(neighbors):
    """Smallest 16-multiple quota per bucket covering the actual input."""
    nb = np.ascontiguousarray(neighbors, dtype=np.int64).reshape(
        N_CORES * TILES, P * S
    )
    quotas = []
    for k in range(4):
        cnts = ((nb >= WIN[k][0]) & (nb < WIN[k][1])).sum(axis=1)
        q = int(-(-max(1, cnts.max()) // 16) * 16)
        quotas.append(q)
    return tuple(quotas)


def _prep_inputs(neighbors, relations, features, weight, relation_weight, QUOTA):
    import ml_dtypes

    NCHUNK = [-(-q // P) for q in QUOTA]
    CHUNKS = sum(NCHUNK)
    IDXCOLS = sum(QUOTA) // 16

    bf16 = ml_dtypes.bfloat16
    inv_s = np.float32(1.0 / S)

    nb = np.ascontiguousarray(neighbors, dtype=np.int64).reshape(N_CORES, TILES, P, S)
    rl = np.ascontiguousarray(relations, dtype=np.int64).reshape(
        N_CORES, TILES * P, S
    )
    feat = np.ascontiguousarray(features.astype(bf16))
    rwts_f = np.zeros((2 * P, D), dtype=np.float32)
    rwts_f[:NUM_REL] = relation_weight.T.astype(np.float32) * inv_s
    rwts = np.ascontiguousarray(rwts_f.astype(bf16))
    wT = np.ascontiguousarray((weight.T.astype(np.float32) * inv_s).astype(bf16))
    iota = np.ascontiguousarray(
        np.tile(np.arange(P, dtype=np.float32), (P, CHUNKS)).astype(bf16)
    )

    in_maps = []
    for core in range(N_CORES):
        nidx = np.zeros((P, TILES * IDXCOLS), dtype=np.int16)
        owner = np.full((P, TILES * CHUNKS), DEAD, dtype=np.float32)
        for t in range(TILES):
            idxs = nb[core, t].ravel()  # j = b*16+s
            owners_flat = np.repeat(np.arange(P), S)
            goff = 0
            cbase = 0
            for k in range(4):
                m = (idxs >= WIN[k][0]) & (idxs < WIN[k][1])
                li = idxs[m] - WIN[k][0]
                lo = owners_flat[m]
                order = np.argsort(li, kind="stable")  # ascending HBM addresses
                li = li[order]
                lo = lo[order]
                cnt_k = len(li)
                assert cnt_k <= QUOTA[k], f"bucket {k} overflow: {cnt_k} > {QUOTA[k]}"
                w = QUOTA[k] // 16
                lpad = np.zeros(QUOTA[k], dtype=np.int16)
                lpad[:cnt_k] = li
                c0 = t * IDXCOLS + goff
                nidx[:, c0 : c0 + w] = _wrap16(lpad, w)
                goff += w
                # owner per slot: slot i -> (p=i%128, chunk=i//128)
                ow = np.full(NCHUNK[k] * P, DEAD, dtype=np.float32)
                ow[:cnt_k] = lo
                owner[
                    :, t * CHUNKS + cbase : t * CHUNKS + cbase + NCHUNK[k]
                ] = ow.reshape(NCHUNK[k], P).T
                cbase += NCHUNK[k]
        # relation histogram: cnt[p, t*256 + k*128 + b] = #{s: rl[t*128+b, s] == k*128+p}
        flat = (
            np.arange(TILES * P, dtype=np.int64)[:, None] * (NUM_REL + 1)
            + rl[core]
        ).ravel()
        counts = np.bincount(flat, minlength=TILES * P * (NUM_REL + 1)).reshape(
            TILES * P, NUM_REL + 1
        )
        arrp = np.zeros((TILES, P, 2 * P), dtype=np.float32)
        arrp[:, :, : NUM_REL + 1] = counts.reshape(TILES, P, NUM_REL + 1)
        cnt_host = (
            arrp.reshape(TILES, P, 2, P)
            .transpose(3, 0, 2, 1)
            .reshape(P, TILES * 2 * P)
        )
        in_maps.append(
            {
                "feat": feat,
                "rwts": rwts,
                "wT": wT,
                "nidx": nidx,
                "owner": np.ascontiguousarray(owner.astype(bf16)),
                "iota": iota,
                "cnt": np.ascontiguousarray(cnt_host.astype(bf16)),
            }
        )
    return in_maps


def run(in_maps, QUOTA, trace=False, tmpdir=None):
    from concourse.bass_utils import run_bass_kernel_spmd

    nc = _get_nc(QUOTA)
    res = run_bass_kernel_spmd(
        nc, in_maps, core_ids=list(range(N_CORES)), trace=trace, tmpdir=tmpdir
    )
    out = np.concatenate([res.results[i]["out"] for i in range(N_CORES)], axis=0)
    return out, res


def kernel(neighbors, relations, features, weight, relation_weight):
    neighbors = np.ascontiguousarray(neighbors)
    relations = np.ascontiguousarray(relations)
    perm = _balance_perm(neighbors)
    QUOTA = _quotas_for(neighbors[perm])
    in_maps = _prep_inputs(
        neighbors[perm], relations[perm], features, weight, relation_weight, QUOTA
    )
    out, _ = run(in_maps, QUOTA, trace=False)
    inv = np.empty(B, dtype=np.int64)
    inv[perm] = np.arange(B)
    return out[inv]
